# revision 1
# baseline (speedup 1.0000x reference)
"""Trainium2 Bass kernel for nn_AutoEncIndex_33887291965861 (topk_masking).

Reference computation:
    soft  = softmax((mat + noise) / temperature)            [training w/ gumbel]
    index = top_k(soft, J).indices                          (full descending sort)
    sel   = greedy row-by-row assignment (first J rows pick the best
            still-unused joint; later rows pick their argmax)
    out   = stop_grad(one_hot(sel)) - stop_grad(mat) + mat

Key facts used here:
  * (0 - m) + m == +0.0 exactly in IEEE fp32, so the output is an exact
    one-hot matrix except the selected entry is (1 - m) + m which is within
    1-2 ulp of 1.0.  Emitting exactly 1.0 keeps the total relative error
    at ~2e-7.
  * softmax and /temperature are strictly monotone per row, so the selection
    order is the order of w = mat + noise (fp32), with lowest-index
    tie-breaking (lax.top_k semantics == vector-engine max_index semantics).
  * The greedy pass over the first J rows selects, for row r, the
    still-available joint with the highest w[r] value (proof: the first
    available joint in row r's descending order always sits within the
    first r+1 positions by pigeonhole, which is exactly the cols<=r window
    the reference uses).  Rows >= J just take their argmax.

Device kernel (SPMD over 8 cores, row-sharded, 4096 rows/core):
  stream 4 MB chunks (512 rows of [mat;noise] as 128 partitions x 4 row
  segments), w = mat + noise on the vector engine, per-segment argmax via
  max/max_index, one-hot built on the scalar engine as Relu(1 - |iota - idx|)
  and streamed out as uint8 (one-hot is exact in u8; host converts to f32).
  Memory bound: 36 MB of HBM traffic per core (32 in + 4 out);
  measured ~150-215 us steady-state per full pass, at the observed
  ~190-235 GB/s per-core DMA ceiling of this platform.

Host: the inherently-sequential greedy over the first 1024 rows (tiny), then
patch those rows of the gathered output.
"""

import os

import numpy as np

HW = 32768
J = 1024
N_CORES = 8
ROWS_PER_CORE = HW // N_CORES  # 4096
P = 128  # SBUF partitions

_NC_CACHE = {}


def _build_nc(rows_per_core: int, j: int, r: int, onehot_engine: str = "act",
              repeat: int = 1, mode: str = "full", bufs: int = 2,
              out_engine: str = "sync", out_dt: str = "u8", mn_bufs: int = 0):
    """Build the per-core Bass module.

    Input "mn" is [2, rows_per_core, j] fp32 — mat stacked with noise (one
    tensor so each chunk loads with a single DMA instruction / single
    semaphore: TRN2 compute instructions can carry only one sync wait).
    Output "out" is the exact one-hot of the per-row argmax of mat + noise.
    r = rows per partition per chunk (chunk covers 128*r rows).
    """
    import concourse.bacc as bacc
    import concourse.mybir as mybir
    from concourse.tile import TileContext

    chunk_rows = P * r
    assert rows_per_core % chunk_rows == 0, (rows_per_core, chunk_rows)
    n_chunks = rows_per_core // chunk_rows
    f32 = mybir.dt.float32

    # Bacc (not raw Bass): its finalize() runs generate_event_semaphores,
    # which splits multi-sem waits — TRN2 instructions carry at most one.
    nc = bacc.Bacc()
    pack = out_dt == "pack"
    odt = {"f32": f32, "u8": mybir.dt.uint8, "bf16": mybir.dt.bfloat16,
           "pack": mybir.dt.uint8}[out_dt]
    # packed mode: 1024 one-hot bits -> 128 bytes per row (byte idx>>3 holds
    # 1 << (idx & 7)); host unpacks with np.unpackbits(bitorder="little")
    jo = j // 8 if pack else j
    ilv = mode == "ilv"
    if ilv:
        # host pre-interleaves so every chunk DMA reads one fully-contiguous
        # 4 MB block (single HBM stream instead of mat/noise 16 MB apart)
        mn = nc.dram_tensor(
            "mn", [rows_per_core // (P * r), P, 2, r * j], f32,
            kind="ExternalInput")
        mnv = mn[:, :, :, :]
    else:
        mn = nc.dram_tensor("mn", [2, rows_per_core, j], f32, kind="ExternalInput")
        # chunk c, partition p holds rows (c*128 + p)*r .. +r-1
        mnv = mn[:, :, :].rearrange("t (c p r) m -> c p t (r m)", p=P, r=r)
    out = nc.dram_tensor("out", [rows_per_core, jo], odt, kind="ExternalOutput")
    outv = out[:, :].rearrange("(c p r) m -> c p (r m)", p=P, r=r)

    out_dma = {"sync": nc.sync, "scalar": nc.scalar, "gpsimd": nc.gpsimd}[out_engine]
    with TileContext(nc) as tc:
        with (
            tc.tile_pool(name="const", bufs=1) as cpool,
            tc.tile_pool(name="work", bufs=bufs) as pool,
            tc.tile_pool(name="mnp", bufs=mn_bufs or bufs) as mnpool,
            tc.tile_pool(name="small", bufs=3) as spool,
        ):
            iota_i = cpool.tile([P, j], mybir.dt.int32)
            nc.gpsimd.iota(iota_i[:], [[1, j]], channel_multiplier=0)
            iota_f = cpool.tile([P, j], f32)
            nc.vector.tensor_copy(iota_f[:], iota_i[:])

            for c in [c for _ in range(repeat) for c in range(n_chunks)]:
                tmn = mnpool.tile([P, 2, r * j], f32, tag="mn")
                if mode in ("split2", "loadonly2"):
                    # mat half on the SP HWDGE ring, noise half on the ACT ring
                    nc.sync.dma_start(tmn[:, 0, :], mnv[c][:, 0, :])
                    nc.scalar.dma_start(tmn[:, 1, :], mnv[c][:, 1, :])
                else:
                    nc.sync.dma_start(tmn[:, :, :], mnv[c])
                if mode in ("loadonly", "loadonly2"):
                    continue
                if mode == "dmaonly":
                    ot = pool.tile([P, r * j], odt, tag="out")
                    nc.vector.tensor_copy(ot[:], tmn[:, 0, :])
                    out_dma.dma_start(outv[c], ot[:])
                    continue
                w = pool.tile([P, r * j], f32, tag="w")
                nc.vector.tensor_add(w[:], tmn[:, 0, :], tmn[:, 1, :])
                ot = pool.tile([P, r * jo], odt, tag="out")
                mx = spool.tile([P, 8 * r], f32, tag="mx")
                ix = spool.tile([P, 8 * r], mybir.dt.uint32, tag="ix")
                for s in range(r):
                    seg = w[:, s * j : (s + 1) * j]
                    oseg = ot[:, s * jo : (s + 1) * jo]
                    nc.vector.max(mx[:, 8 * s : 8 * s + 8], seg)
                    nc.vector.max_index(ix[:, 8 * s : 8 * s + 8], mx[:, 8 * s : 8 * s + 8], seg)
                    if pack:
                        ixs = ix[:, 8 * s : 8 * s + 1]
                        bi = spool.tile([P, 1], mybir.dt.uint32, tag="bi")
                        nc.vector.tensor_scalar(
                            bi[:], ixs, 3, None, op0=mybir.AluOpType.logical_shift_right)
                        rem = spool.tile([P, 1], mybir.dt.uint32, tag="rem")
                        nc.vector.tensor_scalar(
                            rem[:], ixs, 7, None, op0=mybir.AluOpType.bitwise_and)
                        # v = 2^rem exactly: f32 bit pattern (rem+127) << 23
                        vb = spool.tile([P, 1], mybir.dt.uint32, tag="vb")
                        nc.vector.tensor_scalar(
                            vb[:], rem[:], 127, None, op0=mybir.AluOpType.add)
                        ve = spool.tile([P, 1], mybir.dt.uint32, tag="ve")
                        nc.vector.tensor_scalar(
                            ve[:], vb[:], 23, None,
                            op0=mybir.AluOpType.logical_shift_left)
                        bf = spool.tile([P, 1], f32, tag="bf")
                        nc.vector.tensor_scalar_mul(bf[:], bi[:], 1.0)
                        nc.vector.tensor_scalar(
                            oseg, iota_f[:, :jo], bf[:], ve[:].bitcast(f32),
                            op0=mybir.AluOpType.is_equal,
                            op1=mybir.AluOpType.mult)
                    elif onehot_engine == "act":
                        # one-hot on the scalar engine: Relu(1 - |iota - idx|)
                        ixn = spool.tile([P, 1], f32, tag="ixn")
                        nc.vector.tensor_scalar_mul(ixn[:], ix[:, 8 * s : 8 * s + 1], -1.0)
                        ab = spool.tile([P, j], f32, tag="abs")
                        nc.scalar.activation(
                            ab[:], iota_f[:], mybir.ActivationFunctionType.Abs,
                            bias=ixn[:], scale=1.0,
                        )
                        nc.scalar.activation(
                            oseg, ab[:], mybir.ActivationFunctionType.Relu,
                            bias=1.0, scale=-1.0,
                        )
                    else:
                        # one-hot on the vector engine: (iota == idx), f32 compare
                        ixf = spool.tile([P, 1], f32, tag="ixf")
                        nc.vector.tensor_scalar_mul(ixf[:], ix[:, 8 * s : 8 * s + 1], 1.0)
                        nc.vector.tensor_scalar(
                            oseg, iota_f[:], ixf[:], None,
                            op0=mybir.AluOpType.is_equal,
                        )
                out_dma.dma_start(outv[c], ot[:])
    nc.finalize()
    return nc


def _get_nc(rows_per_core=ROWS_PER_CORE, j=J, r=4, onehot_engine=None, repeat=1,
            mode="full", bufs=2, out_engine="sync", out_dt=None, mn_bufs=0):
    if onehot_engine is None:
        onehot_engine = os.environ.get("KERNEL_ONEHOT", "act")
    if out_dt is None:
        out_dt = os.environ.get("KERNEL_OUT_DT", "u8")
    key = (rows_per_core, j, r, onehot_engine, repeat, mode, bufs, out_engine, out_dt,
           mn_bufs)
    if key not in _NC_CACHE:
        _NC_CACHE[key] = _build_nc(*key)
    return _NC_CACHE[key]


def _greedy_select(w_first: np.ndarray) -> np.ndarray:
    """Sequential greedy: row r takes the available joint with max w[r].

    Equivalent to the reference's scan over descending top-k indices.
    """
    n = w_first.shape[0]
    avail = np.ones(n, dtype=bool)
    sel = np.empty(n, dtype=np.int64)
    neg_inf = np.float32(-np.inf)
    for r in range(n):
        row = np.where(avail, w_first[r], neg_inf)
        s = int(np.argmax(row))
        sel[r] = s
        avail[s] = False
    return sel


_RUNNER_CACHE = {}


def _make_runner(r: int = 4, onehot_engine=None, repeat: int = 1, mode: str = "full",
                 bufs: int = 2, out_engine: str = "sync", out_dt=None, mn_bufs: int = 0):
    """Cached runner around run_bass_kernel_spmd.

    The first call goes through run_bass_kernel_spmd (the supported axon/PJRT
    path); during it we capture the jitted SPMD callable that
    run_bass_via_pjrt builds internally, so subsequent calls (and timing
    loops) reuse the compiled executable instead of re-tracing/re-compiling
    (run_bass_via_pjrt creates a fresh jit closure per invocation).
    """
    key = (r, onehot_engine, repeat, mode, bufs, out_engine, out_dt, mn_bufs)
    if key in _RUNNER_CACHE:
        return _RUNNER_CACHE[key]

    import jax
    from concourse.bass_utils import run_bass_kernel_spmd

    nc = _get_nc(ROWS_PER_CORE, J, r, onehot_engine, repeat, mode, bufs, out_engine,
                 out_dt, mn_bufs)
    state = {"fn": None}

    def runner(mn_global: np.ndarray) -> np.ndarray:
        """mn_global: (2*N_CORES, ROWS_PER_CORE, J) per-core [mat, noise]
        pairs. Returns (HW, J) output."""
        if state["fn"] is None:
            per = mn_global.shape[0] // N_CORES
            in_maps = [{"mn": mn_global[per * k : per * (k + 1)]} for k in range(N_CORES)]
            orig_jit = jax.jit

            def capturing_jit(f, *a, **kw):
                g = orig_jit(f, *a, **kw)
                if "donate_argnums" in kw and kw.get("keep_unused"):
                    state["fn"] = g
                return g

            jax.jit = capturing_jit
            try:
                res = run_bass_kernel_spmd(nc, in_maps, core_ids=list(range(N_CORES)))
            finally:
                jax.jit = orig_jit
            out = np.concatenate([r_["out"] for r_ in res.results], axis=0)
            state["out_np_dtype"] = out.dtype
            state["out_shape"] = out.shape
            return out
        outs = state["fn"](mn_global, np.zeros(state["out_shape"], state["out_np_dtype"]))
        out = outs[0] if isinstance(outs, (tuple, list)) else outs
        return np.asarray(out)

    runner.state = state
    _RUNNER_CACHE[key] = runner
    return runner


def stack_inputs(mat: np.ndarray, noise: np.ndarray) -> np.ndarray:
    """Global (2*N_CORES, ROWS_PER_CORE, J): per-core [mat_shard, noise_shard]
    pairs along axis 0, so a P("core") shard is exactly the NEFF's (2, rows, J)
    "mn" tensor."""
    m3 = mat.reshape(N_CORES, ROWS_PER_CORE, J)
    n3 = noise.reshape(N_CORES, ROWS_PER_CORE, J)
    return np.stack([m3, n3], axis=1).reshape(2 * N_CORES, ROWS_PER_CORE, J)


def stack_inputs_ilv(mat: np.ndarray, noise: np.ndarray, r: int = 4) -> np.ndarray:
    """Interleaved layout: global (N_CORES*n_chunks, P, 2, r*J); every chunk is
    one contiguous 4 MB block on device."""
    nck = ROWS_PER_CORE // (P * r)
    m5 = mat.reshape(N_CORES * nck, P, r * J)
    n5 = noise.reshape(N_CORES * nck, P, r * J)
    return np.ascontiguousarray(np.stack([m5, n5], axis=2))


def run_device(mat: np.ndarray, noise: np.ndarray, r: int = 4, onehot_engine=None):
    """Shard row-wise over 8 cores, run the Bass kernel, gather."""
    runner = _make_runner(r, onehot_engine)
    out = runner(stack_inputs(mat, noise))
    return np.asarray(out)


def kernel(sgt_trans_mat, gumbel_noise, use_gumbel_noise=1, is_training=1,
           temperature=30):
    mat = np.ascontiguousarray(np.asarray(sgt_trans_mat, dtype=np.float32))
    assert mat.shape == (HW, J), mat.shape
    training = bool(int(np.asarray(is_training)))
    use_g = training and bool(int(np.asarray(use_gumbel_noise)))
    if use_g:
        noise = np.ascontiguousarray(np.asarray(gumbel_noise, dtype=np.float32))
    else:
        # selection order falls back to mat itself; temperature never matters
        noise = np.zeros_like(mat)

    out = run_device(mat, noise)
    # device output may be bit-packed/uint8/bf16 (exact for one-hot); f32 it
    if out.shape[1] == J // 8:
        out = np.unpackbits(np.ascontiguousarray(out), axis=1,
                            bitorder="little").astype(np.float32)
    elif out.dtype != np.float32:
        out = out.astype(np.float32)
    elif not out.flags.writeable:
        out = out.copy()

    # Host-side greedy over the first J rows (inherently sequential, tiny),
    # then patch those rows of the output.
    w_first = mat[:J] + noise[:J]  # same IEEE fp32 add as the device
    sel = _greedy_select(w_first)
    out[:J] = 0.0
    out[np.arange(J), sel] = np.float32(1.0)
    return out



# revision 8
# speedup vs baseline: 1.0081x; 1.0081x over previous
"""Trainium2 Bass kernel for nn_AutoEncIndex_33887291965861 (topk_masking).

Reference computation:
    soft  = softmax((mat + noise) / temperature)            [training w/ gumbel]
    index = top_k(soft, J).indices                          (full descending sort)
    sel   = greedy row-by-row assignment (first J rows pick the best
            still-unused joint; later rows pick their argmax)
    out   = stop_grad(one_hot(sel)) - stop_grad(mat) + mat

Key facts used here:
  * (0 - m) + m == +0.0 exactly in IEEE fp32, so the output is an exact
    one-hot matrix except the selected entry is (1 - m) + m which is within
    1-2 ulp of 1.0.  Emitting exactly 1.0 keeps the total relative error
    at ~2e-7.
  * softmax and /temperature are strictly monotone per row, so the selection
    order is the order of w = mat + noise (fp32), with lowest-index
    tie-breaking (lax.top_k semantics == vector-engine max_index semantics).
  * The greedy pass over the first J rows selects, for row r, the
    still-available joint with the highest w[r] value (proof: the first
    available joint in row r's descending order always sits within the
    first r+1 positions by pigeonhole, which is exactly the cols<=r window
    the reference uses).  Rows >= J just take their argmax.

Device kernel (SPMD over 8 cores, row-sharded, 4096 rows/core):
  stream 4 MB chunks (512 rows of [mat;noise] as 128 partitions x 4 row
  segments), w = mat + noise on the vector engine, per-segment argmax via
  max/max_index, one-hot built on the scalar engine as Relu(1 - |iota - idx|)
  and streamed out as uint8 (one-hot is exact in u8; host converts to f32).
  Memory bound: 36 MB of HBM traffic per core (32 in + 4 out);
  measured ~150-215 us steady-state per full pass, at the observed
  ~190-235 GB/s per-core DMA ceiling of this platform.

Host: the inherently-sequential greedy over the first 1024 rows (tiny), then
patch those rows of the gathered output.
"""

import os

import numpy as np

HW = 32768
J = 1024
N_CORES = 8
ROWS_PER_CORE = HW // N_CORES  # 4096
P = 128  # SBUF partitions

_NC_CACHE = {}


def _build_nc(rows_per_core: int, j: int, r: int, onehot_engine: str = "act",
              repeat: int = 1, mode: str = "full", bufs: int = 2,
              out_engine: str = "sync", out_dt: str = "u8", mn_bufs: int = 0):
    """Build the per-core Bass module.

    Input "mn" is [2, rows_per_core, j] fp32 — mat stacked with noise (one
    tensor so each chunk loads with a single DMA instruction / single
    semaphore: TRN2 compute instructions can carry only one sync wait).
    Output "out" is the exact one-hot of the per-row argmax of mat + noise.
    r = rows per partition per chunk (chunk covers 128*r rows).
    """
    import concourse.bacc as bacc
    import concourse.mybir as mybir
    from concourse.tile import TileContext

    chunk_rows = P * r
    assert rows_per_core % chunk_rows == 0, (rows_per_core, chunk_rows)
    n_chunks = rows_per_core // chunk_rows
    f32 = mybir.dt.float32

    # Bacc (not raw Bass): its finalize() runs generate_event_semaphores,
    # which splits multi-sem waits — TRN2 instructions carry at most one.
    nc = bacc.Bacc()
    if mode == "v3":
        return _build_nc_v3(nc, mybir, TileContext, rows_per_core, j, r, repeat,
                            bufs, out_dt, mn_bufs)
    pack = out_dt == "pack"
    odt = {"f32": f32, "u8": mybir.dt.uint8, "bf16": mybir.dt.bfloat16,
           "pack": mybir.dt.uint8}[out_dt]
    # packed mode: 1024 one-hot bits -> 128 bytes per row (byte idx>>3 holds
    # 1 << (idx & 7)); host unpacks with np.unpackbits(bitorder="little")
    jo = j // 8 if pack else j
    ilv = mode == "ilv"
    if ilv:
        # host pre-interleaves so every chunk DMA reads one fully-contiguous
        # 4 MB block (single HBM stream instead of mat/noise 16 MB apart)
        mn = nc.dram_tensor(
            "mn", [rows_per_core // (P * r), P, 2, r * j], f32,
            kind="ExternalInput")
        mnv = mn[:, :, :, :]
    else:
        mn = nc.dram_tensor("mn", [2, rows_per_core, j], f32, kind="ExternalInput")
        # chunk c, partition p holds rows (c*128 + p)*r .. +r-1
        mnv = mn[:, :, :].rearrange("t (c p r) m -> c p t (r m)", p=P, r=r)
    out = nc.dram_tensor("out", [rows_per_core, jo], odt, kind="ExternalOutput")
    outv = out[:, :].rearrange("(c p r) m -> c p (r m)", p=P, r=r)

    out_dma = {"sync": nc.sync, "scalar": nc.scalar, "gpsimd": nc.gpsimd}[out_engine]
    with TileContext(nc) as tc:
        with (
            tc.tile_pool(name="const", bufs=1) as cpool,
            tc.tile_pool(name="work", bufs=bufs) as pool,
            tc.tile_pool(name="mnp", bufs=mn_bufs or bufs) as mnpool,
            tc.tile_pool(name="small", bufs=3) as spool,
        ):
            iota_i = cpool.tile([P, j], mybir.dt.int32)
            nc.gpsimd.iota(iota_i[:], [[1, j]], channel_multiplier=0)
            iota_f = cpool.tile([P, j], f32)
            nc.vector.tensor_copy(iota_f[:], iota_i[:])

            for c in [c for _ in range(repeat) for c in range(n_chunks)]:
                tmn = mnpool.tile([P, 2, r * j], f32, tag="mn")
                if mode in ("split2", "loadonly2"):
                    # mat half on the SP HWDGE ring, noise half on the ACT ring
                    nc.sync.dma_start(tmn[:, 0, :], mnv[c][:, 0, :])
                    nc.scalar.dma_start(tmn[:, 1, :], mnv[c][:, 1, :])
                else:
                    nc.sync.dma_start(tmn[:, :, :], mnv[c])
                if mode in ("loadonly", "loadonly2"):
                    continue
                if mode == "dmaonly":
                    ot = pool.tile([P, r * j], odt, tag="out")
                    nc.vector.tensor_copy(ot[:], tmn[:, 0, :])
                    out_dma.dma_start(outv[c], ot[:])
                    continue
                w = pool.tile([P, r * j], f32, tag="w")
                nc.vector.tensor_add(w[:], tmn[:, 0, :], tmn[:, 1, :])
                ot = pool.tile([P, r * jo], odt, tag="out")
                mx = spool.tile([P, 8 * r], f32, tag="mx")
                ix = spool.tile([P, 8 * r], mybir.dt.uint32, tag="ix")
                for s in range(r):
                    seg = w[:, s * j : (s + 1) * j]
                    oseg = ot[:, s * jo : (s + 1) * jo]
                    nc.vector.max(mx[:, 8 * s : 8 * s + 8], seg)
                    nc.vector.max_index(ix[:, 8 * s : 8 * s + 8], mx[:, 8 * s : 8 * s + 8], seg)
                    if pack:
                        ixs = ix[:, 8 * s : 8 * s + 1]
                        bi = spool.tile([P, 1], mybir.dt.uint32, tag="bi")
                        nc.vector.tensor_scalar(
                            bi[:], ixs, 3, None, op0=mybir.AluOpType.logical_shift_right)
                        rem = spool.tile([P, 1], mybir.dt.uint32, tag="rem")
                        nc.vector.tensor_scalar(
                            rem[:], ixs, 7, None, op0=mybir.AluOpType.bitwise_and)
                        # v = 2^rem exactly: f32 bit pattern (rem+127) << 23
                        vb = spool.tile([P, 1], mybir.dt.uint32, tag="vb")
                        nc.vector.tensor_scalar(
                            vb[:], rem[:], 127, None, op0=mybir.AluOpType.add)
                        ve = spool.tile([P, 1], mybir.dt.uint32, tag="ve")
                        nc.vector.tensor_scalar(
                            ve[:], vb[:], 23, None,
                            op0=mybir.AluOpType.logical_shift_left)
                        bf = spool.tile([P, 1], f32, tag="bf")
                        nc.vector.tensor_scalar_mul(bf[:], bi[:], 1.0)
                        nc.vector.tensor_scalar(
                            oseg, iota_f[:, :jo], bf[:], ve[:].bitcast(f32),
                            op0=mybir.AluOpType.is_equal,
                            op1=mybir.AluOpType.mult)
                    elif onehot_engine == "act":
                        # one-hot on the scalar engine: Relu(1 - |iota - idx|)
                        ixn = spool.tile([P, 1], f32, tag="ixn")
                        nc.vector.tensor_scalar_mul(ixn[:], ix[:, 8 * s : 8 * s + 1], -1.0)
                        ab = spool.tile([P, j], f32, tag="abs")
                        nc.scalar.activation(
                            ab[:], iota_f[:], mybir.ActivationFunctionType.Abs,
                            bias=ixn[:], scale=1.0,
                        )
                        nc.scalar.activation(
                            oseg, ab[:], mybir.ActivationFunctionType.Relu,
                            bias=1.0, scale=-1.0,
                        )
                    else:
                        # one-hot on the vector engine: (iota == idx), f32 compare
                        ixf = spool.tile([P, 1], f32, tag="ixf")
                        nc.vector.tensor_scalar_mul(ixf[:], ix[:, 8 * s : 8 * s + 1], 1.0)
                        nc.vector.tensor_scalar(
                            oseg, iota_f[:], ixf[:], None,
                            op0=mybir.AluOpType.is_equal,
                        )
                out_dma.dma_start(outv[c], ot[:])
    nc.finalize()
    return nc


def _build_nc_v3(nc, mybir, TileContext, rows_per_core, j, r, repeat, bufs,
                 out_dt, mn_bufs):
    """v3: engine-balanced, DMA-roofline layout.

    - sync (SP) HWDGE ring carries ONLY the input chunk DMAs, back-to-back.
    - gpsimd (Pool) does w = mat + noise, freeing DVE.
    - DVE does max / max_index per row segment plus one strided batch of
      index math per chunk (byte index + bit value for the packed one-hot).
    - ACT builds the bit-packed one-hot (128 B/row) as two 128-elem
      activations per segment and pushes the output on its own HWDGE ring.
    Output: packed one-hot bits, u8 [rows, j//8] (host unpackbits), so HBM
    write traffic is j/8 bytes/row instead of j.
    """
    P = 128
    f32 = mybir.dt.float32
    u32 = mybir.dt.uint32
    chunk_rows = P * r
    assert rows_per_core % chunk_rows == 0
    n_chunks = rows_per_core // chunk_rows
    assert out_dt == "pack"
    jo = j // 8

    mn = nc.dram_tensor("mn", [2, rows_per_core, j], f32, kind="ExternalInput")
    mnv = mn[:, :, :].rearrange("t (c p r) m -> c p t (r m)", p=P, r=r)
    out = nc.dram_tensor("out", [rows_per_core, jo], mybir.dt.uint8,
                         kind="ExternalOutput")
    outv = out[:, :].rearrange("(c p r) m -> c p (r m)", p=P, r=r)

    with TileContext(nc) as tc:
        with (
            tc.tile_pool(name="const", bufs=1) as cpool,
            tc.tile_pool(name="work", bufs=bufs) as pool,
            tc.tile_pool(name="mnp", bufs=mn_bufs or bufs) as mnpool,
            tc.tile_pool(name="small", bufs=3) as spool,
        ):
            iota_i = cpool.tile([P, jo], mybir.dt.int32)
            nc.gpsimd.iota(iota_i[:], [[1, jo]], channel_multiplier=0)
            iota_f = cpool.tile([P, jo], f32)
            nc.vector.tensor_copy(iota_f[:], iota_i[:])

            for c in [c for _ in range(repeat) for c in range(n_chunks)]:
                tmn = mnpool.tile([P, 2, r * j], f32, tag="mn")
                nc.sync.dma_start(tmn[:, :, :], mnv[c])
                w = pool.tile([P, r * j], f32, tag="w")
                nc.gpsimd.tensor_add(w[:], tmn[:, 0, :], tmn[:, 1, :])
                ot = pool.tile([P, r * jo], mybir.dt.uint8, tag="out")
                mx = spool.tile([P, 8 * r], f32, tag="mx")
                ix = spool.tile([P, 8, r], u32, tag="ix")
                for s in range(r):
                    seg = w[:, s * j : (s + 1) * j]
                    nc.vector.max(mx[:, 8 * s : 8 * s + 8], seg)
                    nc.vector.max_index(ix[:, :, s], mx[:, 8 * s : 8 * s + 8], seg)
                # batch index math over the r argmax heads (strided [P, r])
                heads = ix[:, 0, :]  # [P, r] u32, stride 8 elems
                bi = spool.tile([P, r], u32, tag="bi")
                nc.vector.tensor_scalar(
                    bi[:], heads, 3, None, op0=mybir.AluOpType.logical_shift_right)
                bif = spool.tile([P, r], f32, tag="bif")
                nc.vector.tensor_scalar_mul(bif[:], bi[:], 1.0)
                rem = spool.tile([P, r], u32, tag="rem")
                nc.vector.tensor_scalar(
                    rem[:], heads, 7, None, op0=mybir.AluOpType.bitwise_and)
                vb = spool.tile([P, r], u32, tag="vb")
                nc.vector.tensor_scalar(
                    vb[:], rem[:], 127, None, op0=mybir.AluOpType.add)
                ve = spool.tile([P, r], u32, tag="ve")
                nc.vector.tensor_scalar(
                    ve[:], vb[:], 23, None, op0=mybir.AluOpType.logical_shift_left)
                nv = spool.tile([P, r], f32, tag="nv")
                nc.vector.tensor_scalar_mul(nv[:], ve[:].bitcast(f32), -1.0)
                for s in range(r):
                    oseg = ot[:, s * jo : (s + 1) * jo]
                    ab = spool.tile([P, jo], f32, tag="abs")
                    nc.scalar.activation(
                        ab[:], iota_f[:], mybir.ActivationFunctionType.Abs,
                        bias=bif[:, s : s + 1], scale=-1.0)
                    nc.scalar.activation(
                        oseg, ab[:], mybir.ActivationFunctionType.Relu,
                        bias=ve[:, s : s + 1].bitcast(f32),
                        scale=nv[:, s : s + 1])
                nc.scalar.dma_start(outv[c], ot[:])
    nc.finalize()
    return nc


def _get_nc(rows_per_core=ROWS_PER_CORE, j=J, r=4, onehot_engine=None, repeat=1,
            mode=None, bufs=2, out_engine="scalar", out_dt=None, mn_bufs=0):
    if mode is None:
        mode = os.environ.get("KERNEL_MODE", "v3")
    if onehot_engine is None:
        onehot_engine = os.environ.get("KERNEL_ONEHOT", "act")
    if out_dt is None:
        out_dt = os.environ.get("KERNEL_OUT_DT", "pack")
    key = (rows_per_core, j, r, onehot_engine, repeat, mode, bufs, out_engine, out_dt,
           mn_bufs)
    if key not in _NC_CACHE:
        _NC_CACHE[key] = _build_nc(*key)
    return _NC_CACHE[key]


def _greedy_select(w_first: np.ndarray) -> np.ndarray:
    """Sequential greedy: row r takes the available joint with max w[r].

    Equivalent to the reference's scan over descending top-k indices.
    """
    n = w_first.shape[0]
    avail = np.ones(n, dtype=bool)
    sel = np.empty(n, dtype=np.int64)
    neg_inf = np.float32(-np.inf)
    for r in range(n):
        row = np.where(avail, w_first[r], neg_inf)
        s = int(np.argmax(row))
        sel[r] = s
        avail[s] = False
    return sel


_RUNNER_CACHE = {}


def _make_runner(r: int = 4, onehot_engine=None, repeat: int = 1, mode: str = None,
                 bufs: int = 2, out_engine: str = "scalar", out_dt=None, mn_bufs: int = 0):
    """Cached runner around run_bass_kernel_spmd.

    The first call goes through run_bass_kernel_spmd (the supported axon/PJRT
    path); during it we capture the jitted SPMD callable that
    run_bass_via_pjrt builds internally, so subsequent calls (and timing
    loops) reuse the compiled executable instead of re-tracing/re-compiling
    (run_bass_via_pjrt creates a fresh jit closure per invocation).
    """
    key = (r, onehot_engine, repeat, mode, bufs, out_engine, out_dt, mn_bufs)
    if key in _RUNNER_CACHE:
        return _RUNNER_CACHE[key]

    import jax
    from concourse.bass_utils import run_bass_kernel_spmd

    nc = _get_nc(ROWS_PER_CORE, J, r, onehot_engine, repeat, mode, bufs, out_engine,
                 out_dt, mn_bufs)
    state = {"fn": None}

    def runner(mn_global: np.ndarray) -> np.ndarray:
        """mn_global: (2*N_CORES, ROWS_PER_CORE, J) per-core [mat, noise]
        pairs. Returns (HW, J) output."""
        if state["fn"] is None:
            per = mn_global.shape[0] // N_CORES
            in_maps = [{"mn": mn_global[per * k : per * (k + 1)]} for k in range(N_CORES)]
            orig_jit = jax.jit

            def capturing_jit(f, *a, **kw):
                g = orig_jit(f, *a, **kw)
                if "donate_argnums" in kw and kw.get("keep_unused"):
                    state["fn"] = g
                return g

            jax.jit = capturing_jit
            try:
                res = run_bass_kernel_spmd(nc, in_maps, core_ids=list(range(N_CORES)))
            finally:
                jax.jit = orig_jit
            out = np.concatenate([r_["out"] for r_ in res.results], axis=0)
            state["out_np_dtype"] = out.dtype
            state["out_shape"] = out.shape
            return out
        outs = state["fn"](mn_global, np.zeros(state["out_shape"], state["out_np_dtype"]))
        out = outs[0] if isinstance(outs, (tuple, list)) else outs
        return np.asarray(out)

    runner.state = state
    _RUNNER_CACHE[key] = runner
    return runner


def stack_inputs(mat: np.ndarray, noise: np.ndarray) -> np.ndarray:
    """Global (2*N_CORES, ROWS_PER_CORE, J): per-core [mat_shard, noise_shard]
    pairs along axis 0, so a P("core") shard is exactly the NEFF's (2, rows, J)
    "mn" tensor."""
    m3 = mat.reshape(N_CORES, ROWS_PER_CORE, J)
    n3 = noise.reshape(N_CORES, ROWS_PER_CORE, J)
    return np.stack([m3, n3], axis=1).reshape(2 * N_CORES, ROWS_PER_CORE, J)


def stack_inputs_ilv(mat: np.ndarray, noise: np.ndarray, r: int = 4) -> np.ndarray:
    """Interleaved layout: global (N_CORES*n_chunks, P, 2, r*J); every chunk is
    one contiguous 4 MB block on device."""
    nck = ROWS_PER_CORE // (P * r)
    m5 = mat.reshape(N_CORES * nck, P, r * J)
    n5 = noise.reshape(N_CORES * nck, P, r * J)
    return np.ascontiguousarray(np.stack([m5, n5], axis=2))


def run_device(mat: np.ndarray, noise: np.ndarray, r: int = 4, onehot_engine=None):
    """Shard row-wise over 8 cores, run the Bass kernel, gather."""
    runner = _make_runner(r, onehot_engine)
    out = runner(stack_inputs(mat, noise))
    return np.asarray(out)


def kernel(sgt_trans_mat, gumbel_noise, use_gumbel_noise=1, is_training=1,
           temperature=30):
    mat = np.ascontiguousarray(np.asarray(sgt_trans_mat, dtype=np.float32))
    assert mat.shape == (HW, J), mat.shape
    training = bool(int(np.asarray(is_training)))
    use_g = training and bool(int(np.asarray(use_gumbel_noise)))
    if use_g:
        noise = np.ascontiguousarray(np.asarray(gumbel_noise, dtype=np.float32))
    else:
        # selection order falls back to mat itself; temperature never matters
        noise = np.zeros_like(mat)

    out = run_device(mat, noise)
    # device output may be bit-packed/uint8/bf16 (exact for one-hot); f32 it
    if out.shape[1] == J // 8:
        out = np.unpackbits(np.ascontiguousarray(out), axis=1,
                            bitorder="little").astype(np.float32)
    elif out.dtype != np.float32:
        out = out.astype(np.float32)
    elif not out.flags.writeable:
        out = out.copy()

    # Host-side greedy over the first J rows (inherently sequential, tiny),
    # then patch those rows of the output.
    w_first = mat[:J] + noise[:J]  # same IEEE fp32 add as the device
    sel = _greedy_select(w_first)
    out[:J] = 0.0
    out[np.arange(J), sel] = np.float32(1.0)
    return out



# revision 21
# speedup vs baseline: 1.4610x; 1.4493x over previous
"""Trainium2 Bass kernel for nn_AutoEncIndex_33887291965861 (topk_masking).

Reference computation:
    soft  = softmax((mat + noise) / temperature)            [training w/ gumbel]
    index = top_k(soft, J).indices                          (full descending sort)
    sel   = greedy row-by-row assignment (first J rows pick the best
            still-unused joint; later rows pick their argmax)
    out   = stop_grad(one_hot(sel)) - stop_grad(mat) + mat

Key facts used here:
  * (0 - m) + m == +0.0 exactly in IEEE fp32, so the output is an exact
    one-hot matrix except the selected entry is (1 - m) + m which is within
    1-2 ulp of 1.0.  Emitting exactly 1.0 keeps the total relative error
    at ~2e-7.
  * softmax and /temperature are strictly monotone per row, so the selection
    order is the order of w = mat + noise (fp32), with lowest-index
    tie-breaking (lax.top_k semantics == vector-engine max_index semantics).
  * The greedy pass over the first J rows selects, for row r, the
    still-available joint with the highest w[r] value (proof: the first
    available joint in row r's descending order always sits within the
    first r+1 positions by pigeonhole, which is exactly the cols<=r window
    the reference uses).  Rows >= J just take their argmax.

Device kernel (SPMD over 8 cores, row-sharded, 4096 rows/core):
  stream 4 MB chunks (512 rows of [mat;noise] as 128 partitions x 4 row
  segments), w = mat + noise on the vector engine, per-segment argmax via
  max/max_index, one-hot built on the scalar engine as Relu(1 - |iota - idx|)
  and streamed out as uint8 (one-hot is exact in u8; host converts to f32).
  Memory bound: 36 MB of HBM traffic per core (32 in + 4 out);
  measured ~150-215 us steady-state per full pass, at the observed
  ~190-235 GB/s per-core DMA ceiling of this platform.

Host: the inherently-sequential greedy over the first 1024 rows (tiny), then
patch those rows of the gathered output.
"""

import os

import numpy as np

HW = 32768
J = 1024
N_CORES = 8
ROWS_PER_CORE = HW // N_CORES  # 4096
P = 128  # SBUF partitions

_NC_CACHE = {}


def _build_nc(rows_per_core: int, j: int, r: int, onehot_engine: str = "act",
              repeat: int = 1, mode: str = "full", bufs: int = 2,
              out_engine: str = "sync", out_dt: str = "u8", mn_bufs: int = 0):
    """Build the per-core Bass module.

    Input "mn" is [2, rows_per_core, j] fp32 — mat stacked with noise (one
    tensor so each chunk loads with a single DMA instruction / single
    semaphore: TRN2 compute instructions can carry only one sync wait).
    Output "out" is the exact one-hot of the per-row argmax of mat + noise.
    r = rows per partition per chunk (chunk covers 128*r rows).
    """
    import concourse.bacc as bacc
    import concourse.mybir as mybir
    from concourse.tile import TileContext

    chunk_rows = P * r
    assert rows_per_core % chunk_rows == 0, (rows_per_core, chunk_rows)
    n_chunks = rows_per_core // chunk_rows
    f32 = mybir.dt.float32

    # Bacc (not raw Bass): its finalize() runs generate_event_semaphores,
    # which splits multi-sem waits — TRN2 instructions carry at most one.
    nc = bacc.Bacc()
    if mode in ("v3", "v4", "v4a"):
        return _build_nc_v3(nc, mybir, TileContext, rows_per_core, j, r, repeat,
                            bufs, out_dt, mn_bufs, mode)
    if mode in ("v5", "v5l"):
        return _build_nc_v5(nc, mybir, TileContext, rows_per_core, j, r, repeat,
                            bufs, out_dt, mn_bufs, mode)
    pack = out_dt == "pack"
    odt = {"f32": f32, "u8": mybir.dt.uint8, "bf16": mybir.dt.bfloat16,
           "pack": mybir.dt.uint8}[out_dt]
    # packed mode: 1024 one-hot bits -> 128 bytes per row (byte idx>>3 holds
    # 1 << (idx & 7)); host unpacks with np.unpackbits(bitorder="little")
    jo = j // 8 if pack else j
    ilv = mode == "ilv"
    if ilv:
        # host pre-interleaves so every chunk DMA reads one fully-contiguous
        # 4 MB block (single HBM stream instead of mat/noise 16 MB apart)
        mn = nc.dram_tensor(
            "mn", [rows_per_core // (P * r), P, 2, r * j], f32,
            kind="ExternalInput")
        mnv = mn[:, :, :, :]
    else:
        mn = nc.dram_tensor("mn", [2, rows_per_core, j], f32, kind="ExternalInput")
        # chunk c, partition p holds rows (c*128 + p)*r .. +r-1
        mnv = mn[:, :, :].rearrange("t (c p r) m -> c p t (r m)", p=P, r=r)
    out = nc.dram_tensor("out", [rows_per_core, jo], odt, kind="ExternalOutput")
    outv = out[:, :].rearrange("(c p r) m -> c p (r m)", p=P, r=r)

    out_dma = {"sync": nc.sync, "scalar": nc.scalar, "gpsimd": nc.gpsimd}[out_engine]
    with TileContext(nc) as tc:
        with (
            tc.tile_pool(name="const", bufs=1) as cpool,
            tc.tile_pool(name="work", bufs=bufs) as pool,
            tc.tile_pool(name="mnp", bufs=mn_bufs or bufs) as mnpool,
            tc.tile_pool(name="small", bufs=3) as spool,
        ):
            iota_i = cpool.tile([P, j], mybir.dt.int32)
            nc.gpsimd.iota(iota_i[:], [[1, j]], channel_multiplier=0)
            iota_f = cpool.tile([P, j], f32)
            nc.vector.tensor_copy(iota_f[:], iota_i[:])

            for c in [c for _ in range(repeat) for c in range(n_chunks)]:
                tmn = mnpool.tile([P, 2, r * j], f32, tag="mn")
                if mode in ("split2", "loadonly2"):
                    # mat half on the SP HWDGE ring, noise half on the ACT ring
                    nc.sync.dma_start(tmn[:, 0, :], mnv[c][:, 0, :])
                    nc.scalar.dma_start(tmn[:, 1, :], mnv[c][:, 1, :])
                else:
                    nc.sync.dma_start(tmn[:, :, :], mnv[c])
                if mode in ("loadonly", "loadonly2"):
                    continue
                if mode == "dmaonly":
                    ot = pool.tile([P, r * j], odt, tag="out")
                    nc.vector.tensor_copy(ot[:], tmn[:, 0, :])
                    out_dma.dma_start(outv[c], ot[:])
                    continue
                w = pool.tile([P, r * j], f32, tag="w")
                nc.vector.tensor_add(w[:], tmn[:, 0, :], tmn[:, 1, :])
                ot = pool.tile([P, r * jo], odt, tag="out")
                mx = spool.tile([P, 8 * r], f32, tag="mx")
                ix = spool.tile([P, 8 * r], mybir.dt.uint32, tag="ix")
                for s in range(r):
                    seg = w[:, s * j : (s + 1) * j]
                    oseg = ot[:, s * jo : (s + 1) * jo]
                    nc.vector.max(mx[:, 8 * s : 8 * s + 8], seg)
                    nc.vector.max_index(ix[:, 8 * s : 8 * s + 8], mx[:, 8 * s : 8 * s + 8], seg)
                    if pack:
                        ixs = ix[:, 8 * s : 8 * s + 1]
                        bi = spool.tile([P, 1], mybir.dt.uint32, tag="bi")
                        nc.vector.tensor_scalar(
                            bi[:], ixs, 3, None, op0=mybir.AluOpType.logical_shift_right)
                        rem = spool.tile([P, 1], mybir.dt.uint32, tag="rem")
                        nc.vector.tensor_scalar(
                            rem[:], ixs, 7, None, op0=mybir.AluOpType.bitwise_and)
                        # v = 2^rem exactly: f32 bit pattern (rem+127) << 23
                        vb = spool.tile([P, 1], mybir.dt.uint32, tag="vb")
                        nc.vector.tensor_scalar(
                            vb[:], rem[:], 127, None, op0=mybir.AluOpType.add)
                        ve = spool.tile([P, 1], mybir.dt.uint32, tag="ve")
                        nc.vector.tensor_scalar(
                            ve[:], vb[:], 23, None,
                            op0=mybir.AluOpType.logical_shift_left)
                        bf = spool.tile([P, 1], f32, tag="bf")
                        nc.vector.tensor_scalar_mul(bf[:], bi[:], 1.0)
                        nc.vector.tensor_scalar(
                            oseg, iota_f[:, :jo], bf[:], ve[:].bitcast(f32),
                            op0=mybir.AluOpType.is_equal,
                            op1=mybir.AluOpType.mult)
                    elif onehot_engine == "act":
                        # one-hot on the scalar engine: Relu(1 - |iota - idx|)
                        ixn = spool.tile([P, 1], f32, tag="ixn")
                        nc.vector.tensor_scalar_mul(ixn[:], ix[:, 8 * s : 8 * s + 1], -1.0)
                        ab = spool.tile([P, j], f32, tag="abs")
                        nc.scalar.activation(
                            ab[:], iota_f[:], mybir.ActivationFunctionType.Abs,
                            bias=ixn[:], scale=1.0,
                        )
                        nc.scalar.activation(
                            oseg, ab[:], mybir.ActivationFunctionType.Relu,
                            bias=1.0, scale=-1.0,
                        )
                    else:
                        # one-hot on the vector engine: (iota == idx), f32 compare
                        ixf = spool.tile([P, 1], f32, tag="ixf")
                        nc.vector.tensor_scalar_mul(ixf[:], ix[:, 8 * s : 8 * s + 1], 1.0)
                        nc.vector.tensor_scalar(
                            oseg, iota_f[:], ixf[:], None,
                            op0=mybir.AluOpType.is_equal,
                        )
                out_dma.dma_start(outv[c], ot[:])
    nc.finalize()
    return nc


def _build_nc_v3(nc, mybir, TileContext, rows_per_core, j, r, repeat, bufs,
                 out_dt, mn_bufs, mode="v3"):
    """v3/v4: engine-balanced, DMA-roofline layout.

    - v3: one 4 MB chunk DMA on the sync (SP) HWDGE ring; output on scalar.
    - v4: mat half on sync ring + noise half on scalar ring (two HWDGE
      queues stream concurrently: HW measures 347 GB/s vs 326 single-ring);
      output via gpsimd SWDGE so it never blocks either input FIFO.
    - v4a: v4 input split but output on the scalar ring (FIFO-risk A/B).
    - gpsimd (Pool) does w = mat + noise, freeing DVE.
    - DVE does max / max_index per row segment plus one strided batch of
      index math per chunk (byte index + bit value for the packed one-hot).
    - ACT builds the bit-packed one-hot (128 B/row) as two 128-elem
      activations per segment.
    Output: packed one-hot bits, u8 [rows, j//8] (host unpackbits), so HBM
    write traffic is j/8 bytes/row instead of j.
    """
    P = 128
    f32 = mybir.dt.float32
    u32 = mybir.dt.uint32
    chunk_rows = P * r
    assert rows_per_core % chunk_rows == 0
    n_chunks = rows_per_core // chunk_rows
    ilv = out_dt == "packi"
    jo = j // 8

    if ilv:
        # host pre-interleaves: each chunk is one contiguous 4 MB HBM span,
        # 32 KB contiguous per partition line
        mn = nc.dram_tensor("mn", [n_chunks, P, 2, r * j], f32,
                            kind="ExternalInput")
        mnv = mn[:, :, :, :]
    else:
        mn = nc.dram_tensor("mn", [2, rows_per_core, j], f32, kind="ExternalInput")
        mnv = mn[:, :, :].rearrange("t (c p r) m -> c p t (r m)", p=P, r=r)
    out = nc.dram_tensor("out", [rows_per_core, jo], mybir.dt.uint8,
                         kind="ExternalOutput")
    outv = out[:, :].rearrange("(c p r) m -> c p (r m)", p=P, r=r)

    with TileContext(nc) as tc:
        with (
            tc.tile_pool(name="const", bufs=1) as cpool,
            tc.tile_pool(name="work", bufs=bufs) as pool,
            tc.tile_pool(name="mnp", bufs=mn_bufs or bufs) as mnpool,
            tc.tile_pool(name="small", bufs=3) as spool,
        ):
            iota_i = cpool.tile([P, jo], mybir.dt.int32)
            nc.gpsimd.iota(iota_i[:], [[1, jo]], channel_multiplier=0)
            iota_f = cpool.tile([P, jo], f32)
            nc.vector.tensor_copy(iota_f[:], iota_i[:])

            split_in = mode in ("v4", "v4a")
            out_dma = nc.gpsimd if mode == "v4" else nc.scalar
            for c in [c for _ in range(repeat) for c in range(n_chunks)]:
                tmn = mnpool.tile([P, 2, r * j], f32, tag="mn")
                if split_in:
                    if ilv:
                        nc.sync.dma_start(tmn[:, 0, :], mnv[c, :, 0, :])
                        nc.scalar.dma_start(tmn[:, 1, :], mnv[c, :, 1, :])
                    else:
                        nc.sync.dma_start(tmn[:, 0, :], mnv[c][:, 0, :])
                        nc.scalar.dma_start(tmn[:, 1, :], mnv[c][:, 1, :])
                elif ilv:
                    nc.sync.dma_start(tmn[:, :, :], mnv[c, :, :, :])
                else:
                    nc.sync.dma_start(tmn[:, :, :], mnv[c])
                w = pool.tile([P, r * j], f32, tag="w")
                nc.gpsimd.tensor_add(w[:], tmn[:, 0, :], tmn[:, 1, :])
                ot = pool.tile([P, r * jo], mybir.dt.uint8, tag="out")
                mx = spool.tile([P, 8 * r], f32, tag="mx")
                ix = spool.tile([P, 8, r], u32, tag="ix")
                for s in range(r):
                    seg = w[:, s * j : (s + 1) * j]
                    nc.vector.max(mx[:, 8 * s : 8 * s + 8], seg)
                    nc.vector.max_index(ix[:, :, s], mx[:, 8 * s : 8 * s + 8], seg)
                # batch index math over the r argmax heads (strided [P, r])
                heads = ix[:, 0, :]  # [P, r] u32, stride 8 elems
                bi = spool.tile([P, r], u32, tag="bi")
                nc.vector.tensor_scalar(
                    bi[:], heads, 3, None, op0=mybir.AluOpType.logical_shift_right)
                bif = spool.tile([P, r], f32, tag="bif")
                nc.vector.tensor_scalar_mul(bif[:], bi[:], 1.0)
                rem = spool.tile([P, r], u32, tag="rem")
                nc.vector.tensor_scalar(
                    rem[:], heads, 7, None, op0=mybir.AluOpType.bitwise_and)
                vb = spool.tile([P, r], u32, tag="vb")
                nc.vector.tensor_scalar(
                    vb[:], rem[:], 127, None, op0=mybir.AluOpType.add)
                ve = spool.tile([P, r], u32, tag="ve")
                nc.vector.tensor_scalar(
                    ve[:], vb[:], 23, None, op0=mybir.AluOpType.logical_shift_left)
                nv = spool.tile([P, r], f32, tag="nv")
                nc.vector.tensor_scalar_mul(nv[:], ve[:].bitcast(f32), -1.0)
                for s in range(r):
                    oseg = ot[:, s * jo : (s + 1) * jo]
                    ab = spool.tile([P, jo], f32, tag="abs")
                    nc.scalar.activation(
                        ab[:], iota_f[:], mybir.ActivationFunctionType.Abs,
                        bias=bif[:, s : s + 1], scale=-1.0)
                    nc.scalar.activation(
                        oseg, ab[:], mybir.ActivationFunctionType.Relu,
                        bias=ve[:, s : s + 1].bitcast(f32),
                        scale=nv[:, s : s + 1])
                out_dma.dma_start(outv[c], ot[:])
    nc.finalize()
    return nc


def _build_nc_v5(nc, mybir, TileContext, rows_per_core, j, r, repeat, bufs,
                 out_dt, mn_bufs, mode="v5"):
    """v5: two-ring input streaming at full rate.

    - mat half of each chunk on the sync (SP) HWDGE ring, noise half on the
      scalar (ACT) ring.  ACT runs NO compute at all, so its ring feeds
      noise chunks back-to-back (the v4 lesson: activations in the ACT
      stream FIFO-block the next noise load).
    - output DMA rides the sync ring DELAYED BY TWO CHUNKS, so its data is
      always already computed when the trigger issues - no FIFO stall.
    - gpsimd (Pool): w = mat + noise.  DVE: max/max_index per segment,
      batched index math, and the bit-packed one-hot bytes via
      (iota == byte_idx) * 2^bit with per-partition scalar operands.
    - v5l: loads only (input-ceiling measurement variant).
    """
    P = 128
    f32 = mybir.dt.float32
    u32 = mybir.dt.uint32
    chunk_rows = P * r
    assert rows_per_core % chunk_rows == 0
    n_chunks = rows_per_core // chunk_rows
    ilv = out_dt == "packi"
    loadonly = mode == "v5l"
    jo = j // 8

    if ilv:
        mn = nc.dram_tensor("mn", [n_chunks, P, 2, r * j], f32,
                            kind="ExternalInput")
    else:
        mn = nc.dram_tensor("mn", [2, rows_per_core, j], f32, kind="ExternalInput")
        mnv = mn[:, :, :].rearrange("t (c p r) m -> c p t (r m)", p=P, r=r)
    out = nc.dram_tensor("out", [rows_per_core, jo], mybir.dt.uint8,
                         kind="ExternalOutput")
    outv = out[:, :].rearrange("(c p r) m -> c p (r m)", p=P, r=r)

    with TileContext(nc) as tc:
        with (
            tc.tile_pool(name="const", bufs=1) as cpool,
            tc.tile_pool(name="work", bufs=bufs) as pool,
            tc.tile_pool(name="mnp", bufs=mn_bufs or bufs) as mnpool,
            tc.tile_pool(name="otp", bufs=4) as opool,
            tc.tile_pool(name="small", bufs=3) as spool,
        ):
            iota_i = cpool.tile([P, jo], mybir.dt.int32)
            nc.gpsimd.iota(iota_i[:], [[1, jo]], channel_multiplier=0)
            iota_f = cpool.tile([P, jo], f32)
            nc.vector.tensor_copy(iota_f[:], iota_i[:])

            pending = []

            def flush_one():
                cc, ot_cc = pending.pop(0)
                nc.sync.dma_start(outv[cc], ot_cc[:])

            for c in [c for _ in range(repeat) for c in range(n_chunks)]:
                tmn = mnpool.tile([P, 2, r * j], f32, tag="mn")
                if ilv:
                    nc.sync.dma_start(tmn[:, 0, :], mn[c, :, 0, :])
                    nc.scalar.dma_start(tmn[:, 1, :], mn[c, :, 1, :])
                else:
                    nc.sync.dma_start(tmn[:, 0, :], mnv[c][:, 0, :])
                    nc.scalar.dma_start(tmn[:, 1, :], mnv[c][:, 1, :])
                if loadonly:
                    continue
                w = pool.tile([P, r * j], f32, tag="w")
                nc.gpsimd.tensor_add(w[:], tmn[:, 0, :], tmn[:, 1, :])
                ot = opool.tile([P, r * jo], mybir.dt.uint8, tag="out")
                mx = spool.tile([P, 8 * r], f32, tag="mx")
                ix = spool.tile([P, 8, r], u32, tag="ix")
                for s in range(r):
                    seg = w[:, s * j : (s + 1) * j]
                    nc.vector.max(mx[:, 8 * s : 8 * s + 8], seg)
                    nc.vector.max_index(ix[:, :, s], mx[:, 8 * s : 8 * s + 8], seg)
                heads = ix[:, 0, :]  # [P, r] u32, stride 8 elems
                bi = spool.tile([P, r], u32, tag="bi")
                nc.vector.tensor_scalar(
                    bi[:], heads, 3, None, op0=mybir.AluOpType.logical_shift_right)
                bif = spool.tile([P, r], f32, tag="bif")
                nc.vector.tensor_scalar_mul(bif[:], bi[:], 1.0)
                rem = spool.tile([P, r], u32, tag="rem")
                nc.vector.tensor_scalar(
                    rem[:], heads, 7, None, op0=mybir.AluOpType.bitwise_and)
                vb = spool.tile([P, r], u32, tag="vb")
                nc.vector.tensor_scalar(
                    vb[:], rem[:], 127, None, op0=mybir.AluOpType.add)
                ve = spool.tile([P, r], u32, tag="ve")
                nc.vector.tensor_scalar(
                    ve[:], vb[:], 23, None, op0=mybir.AluOpType.logical_shift_left)
                for s in range(r):
                    oseg = ot[:, s * jo : (s + 1) * jo]
                    nc.vector.tensor_scalar(
                        oseg, iota_f[:], bif[:, s : s + 1],
                        ve[:, s : s + 1].bitcast(f32),
                        op0=mybir.AluOpType.is_equal,
                        op1=mybir.AluOpType.mult)
                pending.append((c, ot))
                if len(pending) > 2:
                    flush_one()
            while pending:
                flush_one()
    nc.finalize()
    return nc


def _get_nc(rows_per_core=ROWS_PER_CORE, j=J, r=4, onehot_engine=None, repeat=1,
            mode=None, bufs=2, out_engine="scalar", out_dt=None, mn_bufs=0):
    if mode is None:
        mode = os.environ.get("KERNEL_MODE", "v3")
    if onehot_engine is None:
        onehot_engine = os.environ.get("KERNEL_ONEHOT", "act")
    if out_dt is None:
        out_dt = os.environ.get("KERNEL_OUT_DT", "packi")
    key = (rows_per_core, j, r, onehot_engine, repeat, mode, bufs, out_engine, out_dt,
           mn_bufs)
    if key not in _NC_CACHE:
        _NC_CACHE[key] = _build_nc(*key)
    return _NC_CACHE[key]


def _greedy_select(w_first: np.ndarray) -> np.ndarray:
    """Sequential greedy: row r takes the available joint with max w[r].

    Equivalent to the reference's scan over descending top-k indices.
    """
    n = w_first.shape[0]
    avail = np.ones(n, dtype=bool)
    sel = np.empty(n, dtype=np.int64)
    neg_inf = np.float32(-np.inf)
    for r in range(n):
        row = np.where(avail, w_first[r], neg_inf)
        s = int(np.argmax(row))
        sel[r] = s
        avail[s] = False
    return sel


_RUNNER_CACHE = {}


def _make_runner(r: int = 4, onehot_engine=None, repeat: int = 1, mode: str = None,
                 bufs: int = 2, out_engine: str = "scalar", out_dt=None, mn_bufs: int = 0):
    """Cached runner around run_bass_kernel_spmd.

    The first call goes through run_bass_kernel_spmd (the supported axon/PJRT
    path); during it we capture the jitted SPMD callable that
    run_bass_via_pjrt builds internally, so subsequent calls (and timing
    loops) reuse the compiled executable instead of re-tracing/re-compiling
    (run_bass_via_pjrt creates a fresh jit closure per invocation).
    """
    if mode is None:
        mode = os.environ.get("KERNEL_MODE", "v3")
    if out_dt is None:
        out_dt = os.environ.get("KERNEL_OUT_DT", "packi")
    key = (r, onehot_engine, repeat, mode, bufs, out_engine, out_dt, mn_bufs)
    if key in _RUNNER_CACHE:
        return _RUNNER_CACHE[key]

    import jax
    from concourse.bass_utils import run_bass_kernel_spmd

    nc = _get_nc(ROWS_PER_CORE, J, r, onehot_engine, repeat, mode, bufs, out_engine,
                 out_dt, mn_bufs)
    state = {"fn": None}

    def runner(mn_global: np.ndarray) -> np.ndarray:
        """mn_global: (2*N_CORES, ROWS_PER_CORE, J) per-core [mat, noise]
        pairs. Returns (HW, J) output."""
        if state["fn"] is None:
            per = mn_global.shape[0] // N_CORES
            in_maps = [{"mn": mn_global[per * k : per * (k + 1)]} for k in range(N_CORES)]
            orig_jit = jax.jit

            def capturing_jit(f, *a, **kw):
                g = orig_jit(f, *a, **kw)
                if "donate_argnums" in kw and kw.get("keep_unused"):
                    state["fn"] = g
                return g

            jax.jit = capturing_jit
            try:
                res = run_bass_kernel_spmd(nc, in_maps, core_ids=list(range(N_CORES)))
            finally:
                jax.jit = orig_jit
            out = np.concatenate([r_["out"] for r_ in res.results], axis=0)
            state["out_np_dtype"] = out.dtype
            state["out_shape"] = out.shape
            return out
        outs = state["fn"](mn_global, np.zeros(state["out_shape"], state["out_np_dtype"]))
        out = outs[0] if isinstance(outs, (tuple, list)) else outs
        return np.asarray(out)

    runner.state = state
    runner.stack = ((lambda m, n: stack_inputs_ilv(m, n, r))
                    if out_dt == "packi" else stack_inputs)
    _RUNNER_CACHE[key] = runner
    return runner


def stack_inputs(mat: np.ndarray, noise: np.ndarray) -> np.ndarray:
    """Global (2*N_CORES, ROWS_PER_CORE, J): per-core [mat_shard, noise_shard]
    pairs along axis 0, so a P("core") shard is exactly the NEFF's (2, rows, J)
    "mn" tensor."""
    m3 = mat.reshape(N_CORES, ROWS_PER_CORE, J)
    n3 = noise.reshape(N_CORES, ROWS_PER_CORE, J)
    return np.stack([m3, n3], axis=1).reshape(2 * N_CORES, ROWS_PER_CORE, J)


def stack_inputs_ilv(mat: np.ndarray, noise: np.ndarray, r: int = 4) -> np.ndarray:
    """Interleaved layout: global (N_CORES*n_chunks, P, 2, r*J); every chunk is
    one contiguous 4 MB block on device."""
    nck = ROWS_PER_CORE // (P * r)
    m5 = mat.reshape(N_CORES * nck, P, r * J)
    n5 = noise.reshape(N_CORES * nck, P, r * J)
    return np.ascontiguousarray(np.stack([m5, n5], axis=2))


def run_device(mat: np.ndarray, noise: np.ndarray, r: int = 4, onehot_engine=None):
    """Shard row-wise over 8 cores, run the Bass kernel, gather."""
    runner = _make_runner(r, onehot_engine)
    out = runner(runner.stack(mat, noise))
    return np.asarray(out)


def kernel(sgt_trans_mat, gumbel_noise, use_gumbel_noise=1, is_training=1,
           temperature=30):
    mat = np.ascontiguousarray(np.asarray(sgt_trans_mat, dtype=np.float32))
    assert mat.shape == (HW, J), mat.shape
    training = bool(int(np.asarray(is_training)))
    use_g = training and bool(int(np.asarray(use_gumbel_noise)))
    if use_g:
        noise = np.ascontiguousarray(np.asarray(gumbel_noise, dtype=np.float32))
    else:
        # selection order falls back to mat itself; temperature never matters
        noise = np.zeros_like(mat)

    out = run_device(mat, noise)
    # device output may be bit-packed/uint8/bf16 (exact for one-hot); f32 it
    if out.shape[1] == J // 8:
        out = np.unpackbits(np.ascontiguousarray(out), axis=1,
                            bitorder="little").astype(np.float32)
    elif out.dtype != np.float32:
        out = out.astype(np.float32)
    elif not out.flags.writeable:
        out = out.copy()

    # Host-side greedy over the first J rows (inherently sequential, tiny),
    # then patch those rows of the output.
    w_first = mat[:J] + noise[:J]  # same IEEE fp32 add as the device
    sel = _greedy_select(w_first)
    out[:J] = 0.0
    out[np.arange(J), sel] = np.float32(1.0)
    return out

